# revision 16
# baseline (speedup 1.0000x reference)
"""CIN (Compressed Interaction Network) kernel for Trainium2, 8 NeuronCores.

Reference (per sample, F=64 fields, E=64 emb, O=128 filters, 3 layers):
    xk_{l+1}[o, e] = relu( sum_{f,c} W_l[o, f*C+c] * x0[f, e] * xk_l[c, e] )
    pooled_l = sum_e xk_{l+1};  y = concat(pooled) @ Wa.T

Strategy (v2 — DVE-minimal / weight-stationary):
  - Data-parallel over batch: 32 samples/core, J = 32*64 = 2048 free columns
    (b-major, e-minor), processed as two software-pipelined halves of 1024 so
    layer-0 matmuls and layer boundaries of one half hide under the other
    half's DVE stretch.
  - Layer 0 is host-folded: H0 = KhatriRao(x0, x0) on upper-triangle pairs
    (K=2176, 17 K-tiles) is computed on host and streamed from DRAM, so
    layer 0 needs no DVE work at all.
  - Layers 1-2 K-tiles are remapped to (4 fields x 32 channels) per
    128-partition tile: t = 16Q + a, partition p -> f = 4a + (p//16)%4,
    c = 32Q + 16*(p//64) + p%16.  Both TT operands then come from
    partition-replicated tiles built with contiguous partition-doubling
    DMAs only:
      mod[a][p]   = x0[4a + (p//16)%4]   (host 64-row seed + 1 doubling,
                                          reused by both layers)
      xkrep[Q][p] = xk[32Q + 16*(p//64) + p%16]  (2 seed copies + 4
                                          doublings per layer boundary)
    This cuts broadcast DMA from 32MB to ~12MB/core vs 1-field-per-tile.
  - One DVE tensor_tensor per K-tile: h_t = xkrep[Q] * mod[a][half] at
    [128, 1024], unit-stride bf16 SBUF -> 2x_1P mode.  No xk replication on
    ScalarE (the same xkrep feeds all 16 field-groups of a c-quarter).
  - K-outer weight-stationary matmuls: per K-tile one LDWEIGHTS + 2 MMs of
    N=512 into two PSUM banks; PE reorder window hides the weight loads.
  - ScalarE drains PSUM with per-sample 64-col ReLU chunks accumulating the
    pooled sums via accum_out (one pass, no 4x replication).
"""

import sys

if "/opt/trn_rl_repo" not in sys.path:
    sys.path.insert(0, "/opt/trn_rl_repo")

import numpy as np
import ml_dtypes

B, F, E, O = 256, 64, 64, 128
NCORES = 8
BC = B // NCORES          # samples per core
J = BC * E                # free columns per core (2048)
JH = J // 2               # half width (1024)
KT0 = 17                  # layer-0 K-tiles (packed symmetric, 2176)
K0 = KT0 * 128
NT = 64                   # layer-1/2 K-tiles

_BF16 = ml_dtypes.bfloat16
_STATE = {}

_PAIRS = [(f, c) for f in range(F) for c in range(f, F)]
_F_IDX = np.array([p[0] for p in _PAIRS] + [0] * (K0 - len(_PAIRS)), np.int64)
_C_IDX = np.array([p[1] for p in _PAIRS] + [0] * (K0 - len(_PAIRS)), np.int64)

# layer-1/2 K-tile index maps: t = 16Q + a, partition p
_P = np.arange(128)
_F_OF_P = (_P // 16) % 4          # field offset within the 4-field group
_C_OF_P = 16 * (_P // 64) + _P % 16   # channel offset within the 32-c quarter


def _k_of_tp(t):
    """reference K index (f*128 + c) for each partition of K-tile t."""
    Q, a = t // 16, t % 16
    f = 4 * a + _F_OF_P
    c = 32 * Q + _C_OF_P
    return f * 128 + c


def _build_nc():
    import concourse.bass as bass
    import concourse.tile as tile
    import concourse.mybir as mybir
    from concourse import bacc

    dt = mybir.dt
    nc = bacc.Bacc("TRN2", target_bir_lowering=False, debug=False)

    h0pack = nc.dram_tensor("h0pack", [2 * KT0, 128, JH], dt.bfloat16,
                            kind="ExternalInput")
    w0t = nc.dram_tensor("w0t", [128, KT0 * O], dt.bfloat16, kind="ExternalInput")
    w1t = nc.dram_tensor("w1t", [4, 128, 16 * O], dt.bfloat16, kind="ExternalInput")
    w2t = nc.dram_tensor("w2t", [4, 128, 16 * O], dt.bfloat16, kind="ExternalInput")
    modseed = nc.dram_tensor("modseed", [16, 64, J], dt.bfloat16,
                             kind="ExternalInput")
    wa = nc.dram_tensor("wa", [O, 3], dt.float32, kind="ExternalInput")
    y = nc.dram_tensor("y", [1, BC], dt.float32, kind="ExternalOutput")

    Relu = mybir.ActivationFunctionType.Relu

    with tile.TileContext(nc) as tc:
        with (
            tc.tile_pool(name="wpool", bufs=1) as wpool,
            tc.tile_pool(name="modpool", bufs=1) as modpool,
            tc.tile_pool(name="h0pool", bufs=6) as h0pool,
            tc.tile_pool(name="hpool", bufs=8) as hpool,
            tc.tile_pool(name="xkpool", bufs=2) as xkpool,
            tc.tile_pool(name="xrpool", bufs=1) as xrpool,
            tc.tile_pool(name="psa", bufs=2, space="PSUM") as psa,
            tc.tile_pool(name="psb", bufs=2, space="PSUM") as psb,
        ):
            # ---- resident SBUF tensors -------------------------------------
            w0 = wpool.tile([128, KT0, O], dt.bfloat16, tag="w0", name="w0")
            w1 = wpool.tile([128, NT, O], dt.bfloat16, tag="w1", name="w1")
            w2 = wpool.tile([128, NT, O], dt.bfloat16, tag="w2", name="w2")
            wa_sb = wpool.tile([O, 3], dt.float32, tag="wa", name="wa_sb")
            pooled = [
                wpool.tile([O, BC], dt.float32, tag=f"pool{l}", name=f"pooled{l}")
                for l in range(3)
            ]
            pscr = wpool.tile([128, 512], dt.bfloat16, tag="pscr", name="pscr")
            mods = [
                modpool.tile([128, J], dt.bfloat16, tag=f"mod{a}", name=f"mod{a}")
                for a in range(16)
            ]

            # preload the ACT Relu table set while DMAs ramp
            nc.vector.memset(pscr[:], 0.0)
            nc.scalar.activation(pscr[0:1, 0:1], pscr[0:1, 0:1], Relu)

            # mod seeds 0-3 first (layer-1 Q0 consumes mods in a-order);
            # doublings ride the idle GpSimd SWDGE ring
            for a in range(4):
                nc.sync.dma_start(mods[a][0:64, :], modseed[a])
                nc.gpsimd.dma_start(mods[a][64:128, :], mods[a][0:64, :])
            nc.sync.dma_start(w0[:].rearrange("p t o -> p (t o)"), w0t[:])
            nc.sync.dma_start(wa_sb[:], wa[:])

            psum_pools = {0: psa, 1: psb}
            acc = {}     # (half, layer) -> [acc_b0, acc_b1]
            xk_sb = {}   # (half, layer) -> SBUF bf16 [128, JH]
            xkrep = {}   # (half, Q) -> [128, JH] (ring reused across layers)
            for h in range(2):
                for Q in range(4):
                    xkrep[(h, Q)] = xrpool.tile(
                        [128, JH], dt.bfloat16, tag=f"xr{h}{Q}", name=f"xr{h}_{Q}"
                    )

            def alloc_acc(h, l):
                pool = psum_pools[h]
                tags = ("a0", "a1") if h == 0 else ("b0", "b1")
                acc[(h, l)] = [
                    pool.tile([128, 512], dt.float32, tag=tags[b],
                              name=f"acc{h}_{l}_{b}")
                    for b in range(2)
                ]

            def emit_l0_tile(h, t):
                h0t = h0pool.tile([128, JH], dt.bfloat16, tag="h0",
                                  name=f"h0_{h}_{t}")
                nc.sync.dma_start(h0t[:], h0pack[KT0 * h + t])
                for b in range(2):
                    nc.tensor.matmul(
                        acc[(h, 0)][b][:], w0[:, t, :],
                        h0t[:, 512 * b: 512 * (b + 1)],
                        start=(t == 0), stop=(t == KT0 - 1),
                    )

            def emit_relu(h, l):
                """Critical-path drain: two wide ReLU ACTs into xk_sb[(h, l)]."""
                xk = xkpool.tile([128, JH], dt.bfloat16, tag=f"xk{h}",
                                 name=f"xk{h}_{l}")
                xk_sb[(h, l)] = xk
                for b in range(2):
                    nc.scalar.activation(
                        xk[:, 512 * b: 512 * (b + 1)], acc[(h, l)][b][:], Relu
                    )

            def emit_pool(h, l):
                """Off-critical pooling: per-sample accum chunks from PSUM."""
                for b in range(2):
                    for s in range(8):
                        col = 16 * h + 8 * b + s
                        nc.scalar.activation(
                            pscr[:, 64 * s: 64 * (s + 1)],
                            acc[(h, l)][b][:, 64 * s: 64 * (s + 1)],
                            Relu,
                            accum_out=pooled[l][:, col: col + 1],
                        )

            def emit_xkrep(h, l):
                """Build xkrep[(h, Q)] from xk_sb[(h, l)].

                One broadcast-source DMA per 64-partition block (stride-0
                re-read of 16 xk rows) — no chaining, so the boundary only
                pays one DMA completion latency.  Q0/Q1 ride the scalar
                HWDGE ring (consumed first); Q2/Q3 ride the GpSimd ring.
                """
                xk = xk_sb[(h, l)]
                for Q in range(4):
                    xr = xkrep[(h, Q)]
                    nc.scalar.dma_start(xr[0:16, :], xk[32 * Q: 32 * Q + 16, :])
                    nc.scalar.dma_start(xr[16:32, :], xr[0:16, :])
                    nc.scalar.dma_start(xr[32:64, :], xr[0:32, :])
                    nc.scalar.dma_start(xr[64:80, :],
                                        xk[32 * Q + 16: 32 * Q + 32, :])
                    nc.scalar.dma_start(xr[80:96, :], xr[64:80, :])
                    nc.scalar.dma_start(xr[96:128, :], xr[64:96, :])

            def emit_l12_tile(h, l, t, w):
                Q, a = t // 16, t % 16
                ht = hpool.tile([128, JH], dt.bfloat16, tag="h",
                                name=f"h{h}_{l}_{t}")
                nc.vector.tensor_tensor(
                    ht[:], xkrep[(h, Q)][:],
                    mods[a][:, JH * h: JH * (h + 1)],
                    op=mybir.AluOpType.mult,
                )
                for b in range(2):
                    nc.tensor.matmul(
                        acc[(h, l)][b][:], w[:, t, :],
                        ht[:, 512 * b: 512 * (b + 1)],
                        start=(t == 0), stop=(t == NT - 1),
                    )

            # ================= emission schedule ============================
            # --- half A layer 0 (h0A streamed from DRAM) --------------------
            alloc_acc(0, 0)
            # dummy matmuls on zeroed scratch: keep the PE busy from ~7us so
            # HAM un-throttles before the real layer-0 matmuls arrive (their
            # group is overwritten by the start=True of the real group)
            for i in range(16):
                nc.tensor.matmul(
                    acc[(0, 0)][0][:], pscr[:, 0:128], pscr[:],
                    start=(i == 0), stop=(i == 15),
                )
            for t in range(KT0):
                emit_l0_tile(0, t)

            # sync queue: w1 chunks / mod seeds / h0B, in required-by order
            nc.sync.dma_start(w1[:, 0:16, :].rearrange("p t o -> p (t o)"), w1t[0])
            # seeds 4-15 ride the (idle) scalar HWDGE ring so they don't
            # stretch the sync ring's issue-rate-bound ramp
            for a in range(4, 16):
                nc.scalar.dma_start(mods[a][0:64, :], modseed[a])
                nc.gpsimd.dma_start(mods[a][64:128, :], mods[a][0:64, :])
            nc.sync.dma_start(w1[:, 16:32, :].rearrange("p t o -> p (t o)"), w1t[1])
            # h0B tiles DMA'd early, consumed by MMs later; bufs=17 keeps all
            # resident so these never block the sync queue
            h0b_tiles = []
            for t in range(8):
                h0t = h0pool.tile([128, JH], dt.bfloat16, tag="h0b",
                                  name=f"h0b_{t}", bufs=KT0)
                nc.sync.dma_start(h0t[:], h0pack[KT0 + t])
                h0b_tiles.append(h0t)
            nc.sync.dma_start(w1[:, 32:48, :].rearrange("p t o -> p (t o)"), w1t[2])
            for t in range(8, KT0):
                h0t = h0pool.tile([128, JH], dt.bfloat16, tag="h0b",
                                  name=f"h0b_{t}", bufs=KT0)
                nc.sync.dma_start(h0t[:], h0pack[KT0 + t])
                h0b_tiles.append(h0t)
            nc.sync.dma_start(w1[:, 48:64, :].rearrange("p t o -> p (t o)"), w1t[3])
            for c in range(4):
                nc.sync.dma_start(
                    w2[:, 16 * c: 16 * (c + 1), :].rearrange("p t o -> p (t o)"),
                    w2t[c],
                )

            # A boundary 0: critical relu -> xkrep; pooling drains later
            emit_relu(0, 0)
            emit_xkrep(0, 0)
            emit_pool(0, 0)

            # --- half A layer 1, with B layer 0 MMs interleaved into PE queue
            alloc_acc(0, 1)
            alloc_acc(1, 0)
            binsert = {20 + 2 * k: k for k in range(KT0)}  # tiles 20..52
            for t in range(NT):
                emit_l12_tile(0, 1, t, w1)
                if t in binsert:
                    k = binsert[t]
                    for b in range(2):
                        nc.tensor.matmul(
                            acc[(1, 0)][b][:], w0[:, k, :],
                            h0b_tiles[k][:, 512 * b: 512 * (b + 1)],
                            start=(k == 0), stop=(k == KT0 - 1),
                        )

            # B boundary 0 -> B layer 1
            emit_relu(1, 0)
            emit_xkrep(1, 0)
            emit_pool(1, 0)
            alloc_acc(1, 1)
            for t in range(NT):
                emit_l12_tile(1, 1, t, w1)

            # A boundary 1 -> A layer 2
            emit_relu(0, 1)
            emit_xkrep(0, 1)
            emit_pool(0, 1)
            alloc_acc(0, 2)
            for t in range(NT):
                emit_l12_tile(0, 2, t, w2)

            # B boundary 1 -> B layer 2
            emit_relu(1, 1)
            emit_xkrep(1, 1)
            emit_pool(1, 1)
            alloc_acc(1, 2)
            for t in range(NT):
                emit_l12_tile(1, 2, t, w2)

            # final pooling: A via hidden ScalarE chunks, B via wide relu +
            # one DVE segmented reduce (DVE is free at the tail)
            emit_pool(0, 2)
            emit_relu(1, 2)
            nc.vector.tensor_reduce(
                pooled[2][:, 16:32],
                xk_sb[(1, 2)][:].rearrange("p (s e) -> p s e", e=E),
                axis=mybir.AxisListType.X,
                op=mybir.AluOpType.add,
            )

            # --- head: y[b] = sum_l wa[:, l] . pooled[l][:, b] --------------
            yac = psa.tile([128, 512], dt.float32, tag="a1", name="yac")
            for l in range(3):
                nc.tensor.matmul(
                    yac[0:1, 0:BC], wa_sb[:, l: l + 1], pooled[l][:],
                    start=(l == 0), stop=(l == 2),
                )
            y_sb = wpool.tile([1, BC], dt.float32, tag="ysb", name="y_sb")
            nc.scalar.copy(y_sb[:], yac[0:1, 0:BC])
            nc.sync.dma_start(y[:], y_sb[:])

    nc.finalize()
    return nc


def _get_nc():
    if "nc" not in _STATE:
        _STATE["nc"] = _build_nc()
    return _STATE["nc"]


def _pack_w0(W0):
    # fold symmetric (f, c) weight pairs onto f <= c; pad to K0 with zeros
    w = np.asarray(W0, np.float32).reshape(O, F, F)
    wp = np.zeros((O, K0), np.float32)
    k = 0
    for f in range(F):
        wp[:, k] = w[:, f, f]
        k += 1
        n = F - f - 1
        if n:
            wp[:, k: k + n] = w[:, f, f + 1:] + w[:, f + 1:, f]
            k += n
    return wp


def _gather_w12(W):
    """[O, F*C] -> [4, 128, 16*O] chunk-major lhsT layout for the tile map."""
    W = np.asarray(W, np.float32)
    wg = np.empty((128, NT, O), np.float32)
    for t in range(NT):
        wg[:, t, :] = W[:, _k_of_tp(t)].T
    return np.ascontiguousarray(
        wg.reshape(128, 4, 16 * O).transpose(1, 0, 2)
    ).astype(_BF16)


def _w_layout(wt):
    K = wt.shape[0]
    return np.ascontiguousarray(
        wt.reshape(K // 128, 128, O).transpose(1, 0, 2).reshape(128, -1)
    )


def _prep_in_maps(x, W0, W1, W2, Wa):
    x = np.asarray(x, dtype=np.float32)

    w0t = _w_layout(_pack_w0(W0).T).astype(_BF16)
    w1t = _gather_w12(W1)
    w2t = _gather_w12(W2)
    wa = np.ascontiguousarray(np.asarray(Wa, np.float32).reshape(3, O).T)

    seedrow = np.repeat(np.arange(4), 16)          # s -> (s//16)%4
    in_maps = []
    for c in range(NCORES):
        xc = x[c * BC: (c + 1) * BC]               # (BC, F, E)
        x0 = np.ascontiguousarray(xc.transpose(1, 0, 2).reshape(F, J))
        x0b = x0.astype(_BF16)

        g = (x0[_F_IDX] * x0[_C_IDX]).astype(_BF16)          # (K0, J)
        h0pack = np.ascontiguousarray(
            g.reshape(KT0, 128, 2, JH).transpose(2, 0, 1, 3).reshape(
                2 * KT0, 128, JH)
        )
        modseed = np.ascontiguousarray(
            x0b[(4 * np.arange(16)[:, None] + seedrow[None, :])]
        )  # (16, 64, J)

        in_maps.append(
            {
                "h0pack": h0pack,
                "w0t": w0t,
                "w1t": w1t,
                "w2t": w2t,
                "modseed": modseed,
                "wa": wa,
            }
        )
    return in_maps


def emulate_core(x, W0, W1, W2, Wa, core):
    """numpy emulation of the device dataflow (fp32) for index-map checks."""
    x = np.asarray(x, np.float32)
    xc = x[core * BC: (core + 1) * BC]
    x0 = xc.transpose(1, 0, 2).reshape(F, J)
    h0 = x0[_F_IDX] * x0[_C_IDX]
    w0p = _pack_w0(W0)
    z = w0p @ h0
    pooled = []
    xk = np.maximum(z, 0.0)
    pooled.append(xk.reshape(O, BC, E).sum(-1))
    for W in (np.asarray(W1, np.float32), np.asarray(W2, np.float32)):
        z = np.zeros((O, J), np.float32)
        for t in range(NT):
            k = _k_of_tp(t)
            Q, a = t // 16, t % 16
            mod = x0[4 * a + _F_OF_P]            # [128, J]
            xr = xk[32 * Q + _C_OF_P]            # [128, J]
            z += W[:, k] @ (mod * xr)
        xk = np.maximum(z, 0.0)
        pooled.append(xk.reshape(O, BC, E).sum(-1))
    feats = np.concatenate(pooled, axis=0)        # (3*O, BC)
    wa = np.asarray(Wa, np.float32).reshape(3 * O)
    return wa @ feats


def _run(inputs, trace=False, **kwargs):
    from concourse.bass_utils import run_bass_kernel_spmd

    nc = _get_nc()
    in_maps = _prep_in_maps(**inputs)
    res = run_bass_kernel_spmd(
        nc, in_maps, core_ids=list(range(NCORES)), trace=trace, **kwargs
    )
    y = np.concatenate(
        [np.asarray(r["y"], np.float32).reshape(BC) for r in res.results]
    )
    return y, res


def kernel(**inputs) -> np.ndarray:
    y, _ = _run(inputs, trace=False)
    return y


# revision 20
# speedup vs baseline: 1.0236x; 1.0236x over previous
"""CIN (Compressed Interaction Network) kernel for Trainium2, 8 NeuronCores.

Reference (per sample, F=64 fields, E=64 emb, O=128 filters, 3 layers):
    xk_{l+1}[o, e] = relu( sum_{f,c} W_l[o, f*C+c] * x0[f, e] * xk_l[c, e] )
    pooled_l = sum_e xk_{l+1};  y = concat(pooled) @ Wa.T

Strategy (v2 — DVE-minimal / weight-stationary):
  - Data-parallel over batch: 32 samples/core, J = 32*64 = 2048 free columns
    (b-major, e-minor), processed as two software-pipelined halves of 1024 so
    layer-0 matmuls and layer boundaries of one half hide under the other
    half's DVE stretch.
  - Layer 0 is host-folded: H0 = KhatriRao(x0, x0) on upper-triangle pairs
    (K=2176, 17 K-tiles) is computed on host and streamed from DRAM, so
    layer 0 needs no DVE work at all.
  - Layers 1-2 K-tiles are remapped to (4 fields x 32 channels) per
    128-partition tile: t = 16Q + a, partition p -> f = 4a + (p//16)%4,
    c = 32Q + 16*(p//64) + p%16.  Both TT operands then come from
    partition-replicated tiles built with contiguous partition-doubling
    DMAs only:
      mod[a][p]   = x0[4a + (p//16)%4]   (host 64-row seed + 1 doubling,
                                          reused by both layers)
      xkrep[Q][p] = xk[32Q + 16*(p//64) + p%16]  (2 seed copies + 4
                                          doublings per layer boundary)
    This cuts broadcast DMA from 32MB to ~12MB/core vs 1-field-per-tile.
  - One DVE tensor_tensor per K-tile: h_t = xkrep[Q] * mod[a][half] at
    [128, 1024], unit-stride bf16 SBUF -> 2x_1P mode.  No xk replication on
    ScalarE (the same xkrep feeds all 16 field-groups of a c-quarter).
  - K-outer weight-stationary matmuls: per K-tile one LDWEIGHTS + 2 MMs of
    N=512 into two PSUM banks; PE reorder window hides the weight loads.
  - ScalarE drains PSUM with per-sample 64-col ReLU chunks accumulating the
    pooled sums via accum_out (one pass, no 4x replication).
"""

import sys

if "/opt/trn_rl_repo" not in sys.path:
    sys.path.insert(0, "/opt/trn_rl_repo")

import numpy as np
import ml_dtypes

B, F, E, O = 256, 64, 64, 128
NCORES = 8
BC = B // NCORES          # samples per core
J = BC * E                # free columns per core (2048)
JH = J // 2               # half width (1024)
KT0 = 17                  # layer-0 K-tiles (packed symmetric, 2176)
K0 = KT0 * 128
NT = 64                   # layer-1/2 K-tiles

_BF16 = ml_dtypes.bfloat16
_FP8 = ml_dtypes.float8_e4m3fn
W0SCL = 1.0
_STATE = {}

_PAIRS = [(f, c) for f in range(F) for c in range(f, F)]
_F_IDX = np.array([p[0] for p in _PAIRS] + [0] * (K0 - len(_PAIRS)), np.int64)
_C_IDX = np.array([p[1] for p in _PAIRS] + [0] * (K0 - len(_PAIRS)), np.int64)

# layer-1/2 K-tile index maps: t = 16Q + a, partition p
_P = np.arange(128)
_F_OF_P = (_P // 16) % 4          # field offset within the 4-field group
_C_OF_P = 16 * (_P // 64) + _P % 16   # channel offset within the 32-c quarter


def _k_of_tp(t):
    """reference K index (f*128 + c) for each partition of K-tile t."""
    Q, a = t // 16, t % 16
    f = 4 * a + _F_OF_P
    c = 32 * Q + _C_OF_P
    return f * 128 + c


def _build_nc():
    import concourse.bass as bass
    import concourse.tile as tile
    import concourse.mybir as mybir
    from concourse import bacc

    dt = mybir.dt
    nc = bacc.Bacc("TRN2", target_bir_lowering=False, debug=False)

    h0pack = nc.dram_tensor("h0pack", [2 * KT0, 128, JH], dt.bfloat16,
                            kind="ExternalInput")
    w0t = nc.dram_tensor("w0t", [128, KT0 * O], dt.bfloat16, kind="ExternalInput")
    w1t = nc.dram_tensor("w1t", [4, 128, 16 * O], dt.bfloat16, kind="ExternalInput")
    w2t = nc.dram_tensor("w2t", [4, 128, 16 * O], dt.bfloat16, kind="ExternalInput")
    modseed = nc.dram_tensor("modseed", [16, 64, J], dt.bfloat16,
                             kind="ExternalInput")
    wa = nc.dram_tensor("wa", [O, 3], dt.float32, kind="ExternalInput")
    y = nc.dram_tensor("y", [1, BC], dt.float32, kind="ExternalOutput")

    Relu = mybir.ActivationFunctionType.Relu

    with tile.TileContext(nc) as tc:
        with (
            tc.tile_pool(name="wpool", bufs=1) as wpool,
            tc.tile_pool(name="modpool", bufs=1) as modpool,
            tc.tile_pool(name="h0pool", bufs=6) as h0pool,
            tc.tile_pool(name="hpool", bufs=8) as hpool,
            tc.tile_pool(name="xkpool", bufs=2) as xkpool,
            tc.tile_pool(name="xrpool", bufs=1) as xrpool,
            tc.tile_pool(name="psa", bufs=2, space="PSUM") as psa,
            tc.tile_pool(name="psb", bufs=2, space="PSUM") as psb,
        ):
            # ---- resident SBUF tensors -------------------------------------
            w0 = wpool.tile([128, KT0, O], dt.bfloat16, tag="w0", name="w0")
            w1 = wpool.tile([128, NT, O], dt.bfloat16, tag="w1", name="w1")
            w2 = wpool.tile([128, NT, O], dt.bfloat16, tag="w2", name="w2")
            wa_sb = wpool.tile([O, 3], dt.float32, tag="wa", name="wa_sb")
            pooled = [
                wpool.tile([O, BC], dt.float32, tag=f"pool{l}", name=f"pooled{l}")
                for l in range(3)
            ]
            pscr = wpool.tile([128, 512], dt.bfloat16, tag="pscr", name="pscr")
            mods = [
                modpool.tile([128, J], dt.bfloat16, tag=f"mod{a}", name=f"mod{a}")
                for a in range(16)
            ]

            # preload the ACT Relu table set while DMAs ramp
            nc.vector.memset(pscr[:], 0.0)
            nc.scalar.activation(pscr[0:1, 0:1], pscr[0:1, 0:1], Relu)

            # mod seeds 0-3 first (layer-1 Q0 consumes mods in a-order);
            # doublings ride the idle GpSimd SWDGE ring
            for a in range(4):
                nc.sync.dma_start(mods[a][0:64, :], modseed[a])
                nc.gpsimd.dma_start(mods[a][64:128, :], mods[a][0:64, :])
            nc.sync.dma_start(w0[:].rearrange("p t o -> p (t o)"), w0t[:])
            nc.sync.dma_start(wa_sb[:], wa[:])

            psum_pools = {0: psa, 1: psb}
            acc = {}     # (half, layer) -> [acc_b0, acc_b1]
            xk_sb = {}   # (half, layer) -> SBUF bf16 [128, JH]
            xkrep = {}   # (half, Q) -> [128, JH] (ring reused across layers)
            for h in range(2):
                for Q in range(4):
                    xkrep[(h, Q)] = xrpool.tile(
                        [128, JH], dt.bfloat16, tag=f"xr{h}{Q}", name=f"xr{h}_{Q}"
                    )

            def alloc_acc(h, l):
                pool = psum_pools[h]
                tags = ("a0", "a1") if h == 0 else ("b0", "b1")
                acc[(h, l)] = [
                    pool.tile([128, 512], dt.float32, tag=tags[b],
                              name=f"acc{h}_{l}_{b}")
                    for b in range(2)
                ]

            def emit_l0_tile(h, t):
                h0t = h0pool.tile([128, JH], dt.bfloat16, tag="h0",
                                  name=f"h0_{h}_{t}")
                nc.sync.dma_start(h0t[:], h0pack[KT0 * h + t])
                for b in range(2):
                    nc.tensor.matmul(
                        acc[(h, 0)][b][:], w0[:, t, :],
                        h0t[:, 512 * b: 512 * (b + 1)],
                        start=(t == 0), stop=(t == KT0 - 1),
                    )

            def emit_relu(h, l):
                """Critical-path drain: two wide ReLU ACTs into xk_sb[(h, l)]."""
                xk = xkpool.tile([128, JH], dt.bfloat16, tag=f"xk{h}",
                                 name=f"xk{h}_{l}")
                xk_sb[(h, l)] = xk
                scl = 1.0 / W0SCL if l == 0 else 1.0
                for b in range(2):
                    nc.scalar.activation(
                        xk[:, 512 * b: 512 * (b + 1)], acc[(h, l)][b][:], Relu,
                        scale=scl,
                    )

            def emit_pool(h, l):
                """Off-critical pooling: per-sample accum chunks from PSUM."""
                scl = 1.0 / W0SCL if l == 0 else 1.0
                for b in range(2):
                    for s in range(8):
                        col = 16 * h + 8 * b + s
                        nc.scalar.activation(
                            pscr[:, 64 * s: 64 * (s + 1)],
                            acc[(h, l)][b][:, 64 * s: 64 * (s + 1)],
                            Relu,
                            scale=scl,
                            accum_out=pooled[l][:, col: col + 1],
                        )

            def emit_xkrep(h, l):
                """Build xkrep[(h, Q)] from xk_sb[(h, l)].

                One broadcast-source DMA per 64-partition block (stride-0
                re-read of 16 xk rows) — no chaining, so the boundary only
                pays one DMA completion latency.  Q0/Q1 ride the scalar
                HWDGE ring (consumed first); Q2/Q3 ride the GpSimd ring.
                """
                xk = xk_sb[(h, l)]
                for Q in range(4):
                    # parallel rings: the vector queue's hoisted waits gate
                    # the first TT on ALL chains, so minimize the max
                    eng = nc.scalar if Q < 2 else nc.gpsimd
                    xr = xkrep[(h, Q)]
                    eng.dma_start(xr[0:16, :], xk[32 * Q: 32 * Q + 16, :])
                    eng.dma_start(xr[16:32, :], xr[0:16, :])
                    eng.dma_start(xr[32:64, :], xr[0:32, :])
                    eng.dma_start(xr[64:80, :], xk[32 * Q + 16: 32 * Q + 32, :])
                    eng.dma_start(xr[80:96, :], xr[64:80, :])
                    eng.dma_start(xr[96:128, :], xr[64:96, :])

            def emit_l12_tile(h, l, t, w):
                Q, a = t // 16, t % 16
                ht = hpool.tile([128, JH], dt.bfloat16, tag="h",
                                name=f"h{h}_{l}_{t}")
                nc.vector.tensor_tensor(
                    ht[:], xkrep[(h, Q)][:],
                    mods[a][:, JH * h: JH * (h + 1)],
                    op=mybir.AluOpType.mult,
                )
                for b in range(2):
                    nc.tensor.matmul(
                        acc[(h, l)][b][:], w[:, t, :],
                        ht[:, 512 * b: 512 * (b + 1)],
                        start=(t == 0), stop=(t == NT - 1),
                    )

            # ================= emission schedule ============================
            # --- half A layer 0 (h0A streamed from DRAM) --------------------
            alloc_acc(0, 0)
            # dummy matmuls on zeroed scratch: keep the PE busy from ~7us so
            # HAM un-throttles before the real layer-0 matmuls arrive (their
            # group is overwritten by the start=True of the real group)
            for i in range(16):
                nc.tensor.matmul(
                    acc[(0, 0)][0][:], pscr[:, 0:128], pscr[:],
                    start=(i == 0), stop=(i == 15),
                )
            for t in range(KT0):
                emit_l0_tile(0, t)

            # sync queue: w1 chunks / mod seeds / h0B, in required-by order
            nc.sync.dma_start(w1[:, 0:16, :].rearrange("p t o -> p (t o)"), w1t[0])
            for a in range(4, 16):
                nc.sync.dma_start(mods[a][0:64, :], modseed[a])
                nc.gpsimd.dma_start(mods[a][64:128, :], mods[a][0:64, :])
            nc.sync.dma_start(w1[:, 16:32, :].rearrange("p t o -> p (t o)"), w1t[1])
            # h0B tiles DMA'd early, consumed by MMs later; bufs=17 keeps all
            # resident so these never block the sync queue
            h0b_tiles = []
            for t in range(8):
                h0t = h0pool.tile([128, JH], dt.bfloat16, tag="h0b",
                                  name=f"h0b_{t}", bufs=KT0)
                nc.sync.dma_start(h0t[:], h0pack[KT0 + t])
                h0b_tiles.append(h0t)
            nc.sync.dma_start(w1[:, 32:48, :].rearrange("p t o -> p (t o)"), w1t[2])
            for t in range(8, KT0):
                h0t = h0pool.tile([128, JH], dt.bfloat16, tag="h0b",
                                  name=f"h0b_{t}", bufs=KT0)
                nc.sync.dma_start(h0t[:], h0pack[KT0 + t])
                h0b_tiles.append(h0t)
            nc.sync.dma_start(w1[:, 48:64, :].rearrange("p t o -> p (t o)"), w1t[3])
            for c in range(4):
                nc.sync.dma_start(
                    w2[:, 16 * c: 16 * (c + 1), :].rearrange("p t o -> p (t o)"),
                    w2t[c],
                )

            # A boundary 0: critical relu -> xkrep; pooling drains later
            emit_relu(0, 0)
            emit_xkrep(0, 0)
            emit_pool(0, 0)

            # --- half A layer 1, with B layer 0 MMs interleaved into PE queue
            alloc_acc(0, 1)
            alloc_acc(1, 0)
            binsert = {20 + 2 * k: k for k in range(KT0)}  # tiles 20..52
            for t in range(NT):
                emit_l12_tile(0, 1, t, w1)
                if t in binsert:
                    k = binsert[t]
                    for b in range(2):
                        nc.tensor.matmul(
                            acc[(1, 0)][b][:], w0[:, k, :],
                            h0b_tiles[k][:, 512 * b: 512 * (b + 1)],
                            start=(k == 0), stop=(k == KT0 - 1),
                        )

            # B boundary 0 -> B layer 1
            emit_relu(1, 0)
            emit_xkrep(1, 0)
            emit_pool(1, 0)
            alloc_acc(1, 1)
            for t in range(NT):
                emit_l12_tile(1, 1, t, w1)

            # A boundary 1 -> A layer 2
            emit_relu(0, 1)
            emit_xkrep(0, 1)
            emit_pool(0, 1)
            alloc_acc(0, 2)
            for t in range(NT):
                emit_l12_tile(0, 2, t, w2)

            # B boundary 1 -> B layer 2
            emit_relu(1, 1)
            emit_xkrep(1, 1)
            emit_pool(1, 1)
            alloc_acc(1, 2)
            for t in range(NT):
                emit_l12_tile(1, 2, t, w2)

            # final pooling: A via hidden ScalarE chunks, B via wide relu +
            # one DVE segmented reduce (DVE is free at the tail)
            emit_pool(0, 2)
            emit_relu(1, 2)
            nc.vector.tensor_reduce(
                pooled[2][:, 16:32],
                xk_sb[(1, 2)][:].rearrange("p (s e) -> p s e", e=E),
                axis=mybir.AxisListType.X,
                op=mybir.AluOpType.add,
            )

            # --- head: y[b] = sum_l wa[:, l] . pooled[l][:, b] --------------
            yac = psa.tile([128, 512], dt.float32, tag="a1", name="yac")
            for l in range(3):
                nc.tensor.matmul(
                    yac[0:1, 0:BC], wa_sb[:, l: l + 1], pooled[l][:],
                    start=(l == 0), stop=(l == 2),
                )
            y_sb = wpool.tile([1, BC], dt.float32, tag="ysb", name="y_sb")
            nc.scalar.copy(y_sb[:], yac[0:1, 0:BC])
            nc.sync.dma_start(y[:], y_sb[:])

    nc.finalize()
    return nc


def _get_nc():
    if "nc" not in _STATE:
        _STATE["nc"] = _build_nc()
    return _STATE["nc"]


def _pack_w0(W0):
    # fold symmetric (f, c) weight pairs onto f <= c; pad to K0 with zeros
    w = np.asarray(W0, np.float32).reshape(O, F, F)
    wp = np.zeros((O, K0), np.float32)
    k = 0
    for f in range(F):
        wp[:, k] = w[:, f, f]
        k += 1
        n = F - f - 1
        if n:
            wp[:, k: k + n] = w[:, f, f + 1:] + w[:, f + 1:, f]
            k += n
    return wp


def _gather_w12(W):
    """[O, F*C] -> [4, 128, 16*O] chunk-major lhsT layout for the tile map."""
    W = np.asarray(W, np.float32)
    wg = np.empty((128, NT, O), np.float32)
    for t in range(NT):
        wg[:, t, :] = W[:, _k_of_tp(t)].T
    return np.ascontiguousarray(
        wg.reshape(128, 4, 16 * O).transpose(1, 0, 2)
    ).astype(_BF16)


def _w_layout(wt):
    K = wt.shape[0]
    return np.ascontiguousarray(
        wt.reshape(K // 128, 128, O).transpose(1, 0, 2).reshape(128, -1)
    )


def _prep_in_maps(x, W0, W1, W2, Wa):
    x = np.asarray(x, dtype=np.float32)

    w0t = _w_layout(_pack_w0(W0).T).astype(_BF16)
    w1t = _gather_w12(W1)
    w2t = _gather_w12(W2)
    wa = np.ascontiguousarray(np.asarray(Wa, np.float32).reshape(3, O).T)

    seedrow = np.repeat(np.arange(4), 16)          # s -> (s//16)%4
    in_maps = []
    for c in range(NCORES):
        xc = x[c * BC: (c + 1) * BC]               # (BC, F, E)
        x0 = np.ascontiguousarray(xc.transpose(1, 0, 2).reshape(F, J))
        x0b = x0.astype(_BF16)

        g = (x0[_F_IDX] * x0[_C_IDX]).astype(_BF16)          # (K0, J)
        h0pack = np.ascontiguousarray(
            g.reshape(KT0, 128, 2, JH).transpose(2, 0, 1, 3).reshape(
                2 * KT0, 128, JH)
        )
        modseed = np.ascontiguousarray(
            x0b[(4 * np.arange(16)[:, None] + seedrow[None, :])]
        )  # (16, 64, J)

        in_maps.append(
            {
                "h0pack": h0pack,
                "w0t": w0t,
                "w1t": w1t,
                "w2t": w2t,
                "modseed": modseed,
                "wa": wa,
            }
        )
    return in_maps


def emulate_core(x, W0, W1, W2, Wa, core):
    """numpy emulation of the device dataflow (fp32) for index-map checks."""
    x = np.asarray(x, np.float32)
    xc = x[core * BC: (core + 1) * BC]
    x0 = xc.transpose(1, 0, 2).reshape(F, J)
    h0 = x0[_F_IDX] * x0[_C_IDX]
    w0p = _pack_w0(W0)
    z = w0p @ h0
    pooled = []
    xk = np.maximum(z, 0.0)
    pooled.append(xk.reshape(O, BC, E).sum(-1))
    for W in (np.asarray(W1, np.float32), np.asarray(W2, np.float32)):
        z = np.zeros((O, J), np.float32)
        for t in range(NT):
            k = _k_of_tp(t)
            Q, a = t // 16, t % 16
            mod = x0[4 * a + _F_OF_P]            # [128, J]
            xr = xk[32 * Q + _C_OF_P]            # [128, J]
            z += W[:, k] @ (mod * xr)
        xk = np.maximum(z, 0.0)
        pooled.append(xk.reshape(O, BC, E).sum(-1))
    feats = np.concatenate(pooled, axis=0)        # (3*O, BC)
    wa = np.asarray(Wa, np.float32).reshape(3 * O)
    return wa @ feats


def _run(inputs, trace=False, **kwargs):
    from concourse.bass_utils import run_bass_kernel_spmd

    nc = _get_nc()
    in_maps = _prep_in_maps(**inputs)
    res = run_bass_kernel_spmd(
        nc, in_maps, core_ids=list(range(NCORES)), trace=trace, **kwargs
    )
    y = np.concatenate(
        [np.asarray(r["y"], np.float32).reshape(BC) for r in res.results]
    )
    return y, res


def kernel(**inputs) -> np.ndarray:
    y, _ = _run(inputs, trace=False)
    return y
